# revision 7
# baseline (speedup 1.0000x reference)
"""CoarseMatching (LoFTR-style dual-softmax matching) on 8 Trainium2 cores.

Sharding: each core owns 600 rows (L dim) of both pairs (N=2).  Per pair:
sim = (f0 @ f1^T) / (C^0.5 * C^0.5 * TEMP) computed with fp32r matmuls,
P = exp(sim) kept resident, row sums via ACT accum, column sums via
ones-matmul on PE + one 8-core AllReduce, conf = (P*rsqrt(rsum))^2 * cinv
written back to HBM.  Host assembles shards and applies the (empty in
practice) threshold/border/mutual-NN masking.
"""

import sys
import numpy as np

sys.path.insert(0, "/opt/trn_rl_repo")

import concourse.bacc as bacc
import concourse.bass as bass
import concourse.tile as tile
from concourse import mybir
from concourse.bass_utils import run_bass_kernel_spmd

N_CORES = 8
N, L, S, C = 2, 4800, 4800, 256
H0, W0, H1, W1 = 60, 80, 60, 80
THR = 0.2
BORDER_RM = 2
TEMP = 0.1
SCALE = 1.0 / (C * TEMP)  # folded into f0T: (1/sqrt(C))^2 / TEMP

SHARD = L // N_CORES          # 600 rows per core per pair
LP = 120                      # rows per l-tile (partition dim)
NLT = SHARD // LP             # 5 l-tiles
NBW = 480                     # columns per matmul block
NNB = S // NBW                # 10 column blocks
KC = 128                      # contraction chunk (partitions)
NKC = C // KC                 # 2 chunks

F32 = mybir.dt.float32
F32R = mybir.dt.float32r


def _row_blocks(total):
    blocks = []
    start = 0
    while start < total:
        sz = min(128, total - start)
        blocks.append((start, sz))
        start += sz
    return blocks


def build_nc():
    nc = bacc.Bacc("TRN2", target_bir_lowering=False, num_devices=N_CORES)

    f0s = nc.declare_dram_parameter("f0s", [N, SHARD, C], F32, isOutput=False)
    f1f = nc.declare_dram_parameter("f1f", [N, S, C], F32, isOutput=False)
    ident = nc.declare_dram_parameter("ident", [128, 128], F32, isOutput=False)
    ones_in = nc.declare_dram_parameter("ones_in", [128, 128], F32, isOutput=False)
    conf_o = nc.declare_dram_parameter("conf_o", [N, SHARD, S], F32, isOutput=True)

    with tile.TileContext(nc) as tc:
        with (
            tc.tile_pool(name="single", bufs=1) as single,
            tc.tile_pool(name="f1n", bufs=6) as f1n_pool,
            tc.tile_pool(name="f0n", bufs=2) as f0n_pool,
            tc.tile_pool(name="f1T", bufs=2) as f1T_pool,
            tc.tile_pool(name="f0T", bufs=4) as f0T_pool,
            tc.tile_pool(name="Pp", bufs=5) as P_pool,
            tc.tile_pool(name="stats", bufs=12) as stats_pool,
            tc.tile_pool(name="tiny", bufs=24) as tiny_pool,
            tc.tile_pool(name="cs", bufs=4) as cs_pool,
            tc.tile_pool(name="cb", bufs=1) as cb_pool,
            tc.tile_pool(name="stage", bufs=3) as stage_pool,
            tc.tile_pool(name="pt", bufs=2, space="PSUM") as pt_pool,
            tc.tile_pool(name="ps", bufs=3, space="PSUM") as ps_pool,
            tc.tile_pool(name="pc", bufs=2, space="PSUM") as pc_pool,
            tc.tile_pool(name="dram", bufs=1, space="DRAM") as dram_pool,
        ):
            ident_t = single.tile([128, 128], F32)
            nc.sync.dma_start(out=ident_t, in_=ident[:, :])
            ones_f = single.tile([128, 128], F32)
            nc.sync.dma_start(out=ones_f, in_=ones_in[:, :])
            ones_r = single.tile([128, 128], F32R)
            nc.vector.tensor_copy(ones_r, ones_f)

            cc_in = []
            cc_out = []
            stag = []
            for p in range(N):
                t_in = dram_pool.tile([S], F32, name=f"cc_in{p}", tag=f"cc_in{p}")
                t_out = dram_pool.tile([S], F32, name=f"cc_out{p}", addr_space="Shared", tag=f"cc_out{p}")
                t_st = dram_pool.tile([S], F32, name=f"stag{p}", tag=f"stag{p}")
                cc_in.append(t_in)
                cc_out.append(t_out)
                stag.append(t_st)

            for p in range(N):
                # ---- transpose f0 shard: [SHARD, C] -> f0T[k] [128, SHARD] ----
                f0T = []
                for k in range(NKC):
                    t = f0T_pool.tile([128, SHARD], F32R, name=f"f0T{p}{k}", tag="f0T")
                    f0T.append(t)
                for (rs, sz) in _row_blocks(SHARD):
                    f0nb = f0n_pool.tile([128, C], F32, name="f0nb")
                    nc.sync.dma_start(out=f0nb[:sz, :], in_=f0s[p, rs : rs + sz, :])
                    for k in range(NKC):
                        ptt = pt_pool.tile([128, 512], F32, name="ptt0", tag="ptt")
                        nc.tensor.transpose(
                            ptt[:, :sz],
                            f0nb[:sz, k * KC : (k + 1) * KC],
                            ident_t[:sz, :sz],
                        )
                        # scale folded here; output dtype fp32r rounds
                        nc.scalar.activation(
                            f0T[k][:, rs : rs + sz],
                            ptt[:, :sz],
                            mybir.ActivationFunctionType.Copy,
                            scale=SCALE,
                        )

                # ---- transpose f1: [S, C] -> f1T[k] [128, S] ----
                f1T = []
                for k in range(NKC):
                    t = f1T_pool.tile([128, S], F32R, name=f"f1T{p}{k}", tag="f1T")
                    f1T.append(t)
                f1_blocks = _row_blocks(S)
                for g in range(0, len(f1_blocks), 4):
                    group = f1_blocks[g : g + 4]
                    tiles = []
                    for (rs, sz) in group:
                        f1nb = f1n_pool.tile([128, C], F32, name="f1nb")
                        nc.sync.dma_start(out=f1nb[:sz, :], in_=f1f[p, rs : rs + sz, :])
                        tiles.append((f1nb, rs, sz))
                    for k in range(NKC):
                        ptt = pt_pool.tile([128, 512], F32, name="ptt1", tag="ptt")
                        off = 0
                        for (f1nb, rs, sz) in tiles:
                            nc.tensor.transpose(
                                ptt[:, off : off + sz],
                                f1nb[:sz, k * KC : (k + 1) * KC],
                                ident_t[:sz, :sz],
                            )
                            off += sz
                        gs = group[0][0]
                        nc.vector.tensor_copy(f1T[k][:, gs : gs + off], ptt[:, :off])

                # ---- matmul + exp (phase A) ----
                P_tiles = []
                rsp_tiles = []
                srinv_tiles = []
                for lt in range(NLT):
                    P_lt = P_pool.tile([LP, S], F32R, name=f"P{lt}", tag="P")
                    P_tiles.append(P_lt)
                    rsp = stats_pool.tile([LP, 16], F32, name=f"rsp{lt}", tag="rsp")
                    rsp_tiles.append(rsp)
                    lc = lt * LP
                    for nb in range(NNB):
                        cb0 = nb * NBW
                        pst = ps_pool.tile([LP, NBW], F32, name="pst")
                        nc.tensor.matmul(
                            pst,
                            f0T[0][:, lc : lc + LP],
                            f1T[0][:, cb0 : cb0 + NBW],
                            start=True,
                            stop=False,
                        )
                        nc.tensor.matmul(
                            pst,
                            f0T[1][:, lc : lc + LP],
                            f1T[1][:, cb0 : cb0 + NBW],
                            start=False,
                            stop=True,
                        )
                        nc.scalar.activation(
                            P_lt[:, cb0 : cb0 + NBW],
                            pst,
                            mybir.ActivationFunctionType.Exp,
                            accum_out=rsp[:, nb : nb + 1],
                        )
                    # rsum -> srinv = exp(-0.5 * ln(rsum))
                    rs1 = tiny_pool.tile([LP, 1], F32, name=f"rs1_{lt}", tag="rs1")
                    nc.vector.tensor_reduce(
                        rs1, rsp[:, 0:NNB], axis=mybir.AxisListType.X,
                        op=mybir.AluOpType.add,
                    )
                    ln1 = tiny_pool.tile([LP, 1], F32, name=f"ln1_{lt}", tag="ln1")
                    nc.scalar.activation(
                        ln1, rs1, mybir.ActivationFunctionType.Ln
                    )
                    srinv = tiny_pool.tile([LP, 1], F32, name=f"srinv{lt}", tag="srinv")
                    nc.scalar.activation(
                        srinv, ln1, mybir.ActivationFunctionType.Exp, scale=-0.5
                    )
                    srinv_tiles.append(srinv)

                # ---- csum via ones-matmul, accumulate over l-tiles ----
                for nb in range(NNB):
                    cb0 = nb * NBW
                    pct = pc_pool.tile([128, NBW], F32, name="pct")
                    for lt in range(NLT):
                        nc.tensor.matmul(
                            pct,
                            ones_r[:LP, :],
                            P_tiles[lt][:, cb0 : cb0 + NBW],
                            start=(lt == 0),
                            stop=(lt == NLT - 1),
                        )
                    csb = cs_pool.tile([1, NBW], F32, name="csb")
                    nc.scalar.copy(csb, pct[0:1, :])
                    nc.sync.dma_start(out=cc_in[p][cb0 : cb0 + NBW], in_=csb)

                # ---- all-reduce column sums across the 8 cores ----
                nc.gpsimd.collective_compute(
                    "AllReduce",
                    mybir.AluOpType.add,
                    replica_groups=[list(range(N_CORES))],
                    ins=[cc_in[p][:].opt()],
                    outs=[cc_out[p][:].opt()],
                )

                # ---- cinv = 1/csum, broadcast to all partitions ----
                csr = cs_pool.tile([96, 50], F32, name="csr")
                nc.sync.dma_start(
                    out=csr, in_=cc_out[p][:].rearrange("(a b) -> a b", a=96)
                )
                cis = cs_pool.tile([96, 50], F32, name="cis")
                nc.vector.reciprocal(cis, csr)
                nc.sync.dma_start(
                    out=stag[p][:].rearrange("(a b) -> a b", a=96), in_=cis
                )
                cb_t = cb_pool.tile([128, S], F32, name="cb_t")
                stag_ap = stag[p][:]
                stag_bcast = bass.AP(
                    tensor=stag_ap.tensor,
                    offset=stag_ap.offset,
                    ap=[[0, 128]] + list(stag_ap.ap),
                )
                nc.sync.dma_start(out=cb_t, in_=stag_bcast)

                # ---- phase B: conf = (P * srinv)^2 * cinv ----
                for lt in range(NLT):
                    P_lt = P_tiles[lt]
                    # in-place square with per-partition scale (via f32 view;
                    # the DVE/ISA path does not accept fp32r on ACT in-place)
                    nc.scalar.activation(
                        P_lt,
                        P_lt.bitcast(F32),
                        mybir.ActivationFunctionType.Square,
                        scale=srinv_tiles[lt],
                    )
                    lr = lt * LP
                    for nb in range(NNB):
                        cb0 = nb * NBW
                        st = stage_pool.tile([LP, NBW], F32, name="st")
                        nc.vector.tensor_mul(
                            st,
                            P_lt.bitcast(F32)[:, cb0 : cb0 + NBW],
                            cb_t[:LP, cb0 : cb0 + NBW],
                        )
                        nc.sync.dma_start(
                            out=conf_o[p, lr : lr + LP, cb0 : cb0 + NBW], in_=st
                        )

    nc.compile()
    return nc


_CACHED = {}


def _get_nc():
    if "nc" not in _CACHED:
        _CACHED["nc"] = build_nc()
    return _CACHED["nc"]


def run_device(feat_c0, feat_c1, trace=False, tmpdir=None):
    """Runs the SPMD kernel; returns (conf [N,L,S], rowmax [N,L], results obj)."""
    nc = _get_nc()
    ident = np.eye(128, dtype=np.float32)
    ones = np.ones((128, 128), dtype=np.float32)
    in_maps = []
    for c in range(N_CORES):
        rs = c * SHARD
        in_maps.append(
            {
                "f0s": np.ascontiguousarray(feat_c0[:, rs : rs + SHARD, :]),
                "f1f": np.ascontiguousarray(feat_c1),
                "ident": ident,
                "ones_in": ones,
            }
        )
    res = run_bass_kernel_spmd(
        nc, in_maps, list(range(N_CORES)), trace=trace, tmpdir=tmpdir
    )
    conf = np.empty((N, L, S), dtype=np.float32)
    for c in range(N_CORES):
        rs = c * SHARD
        conf[:, rs : rs + SHARD, :] = res.results[c]["conf_o"]
    return conf, res


def _interior(n, idx):
    return (idx >= BORDER_RM) & (idx < n - BORDER_RM)


def kernel(feat_c0, feat_c1, h0c, w0c, h1c, w1c):
    feat_c0 = np.asarray(feat_c0, dtype=np.float32)
    feat_c1 = np.asarray(feat_c1, dtype=np.float32)
    h0c, w0c, h1c, w1c = int(h0c), int(w0c), int(h1c), int(w1c)
    assert feat_c0.shape == (N, L, C) and feat_c1.shape == (N, S, C)
    assert (h0c * w0c, h1c * w1c) == (L, S)

    conf, _ = run_device(feat_c0, feat_c1)
    rowmax = conf.max(axis=2)

    # ---- host finalize: threshold + border + mutual-NN (tiny) ----
    mask_v = np.zeros((N, L), dtype=bool)
    j_ids = np.zeros((N, L), dtype=np.int32)
    mconf = np.zeros((N, L), dtype=np.float32)

    cand_n, cand_l = np.nonzero(rowmax > THR)
    for n_i, l_i in zip(cand_n, cand_l):
        # row border (interior of the h0c x w0c grid)
        if not (_interior(h0c, l_i // w0c) and _interior(w0c, l_i % w0c)):
            continue
        row = conf[n_i, l_i]
        j = int(np.argmax(row))
        v = row[j]
        if not (v > THR):
            continue
        # column border (interior of the h1c x w1c grid)
        if not (_interior(h1c, j // w1c) and _interior(w1c, j % w1c)):
            continue
        # mutual nearest neighbor: also the max of its column
        if conf[n_i, :, j].max() != v:
            continue
        mask_v[n_i, l_i] = True
        j_ids[n_i, l_i] = j
        mconf[n_i, l_i] = v

    mkpts1_c = np.stack([j_ids % w1c, j_ids // w1c], axis=-1).astype(np.int32)
    return conf, mask_v, j_ids, mkpts1_c, mconf


# revision 11
# speedup vs baseline: 1.1885x; 1.1885x over previous
"""CoarseMatching (LoFTR-style dual-softmax matching) on 8 Trainium2 cores.

Sharding: each core owns 600 rows (L dim) of both pairs (N=2).  Per pair:
sim = (f0 @ f1^T) / (C^0.5 * C^0.5 * TEMP) computed with fp32r matmuls,
P = exp(sim) kept resident, row sums via ACT accum, column sums via
ones-matmul on PE + one 8-core AllReduce, conf = (P*rsqrt(rsum))^2 * cinv
written back to HBM.  Host assembles shards and applies the (empty in
practice) threshold/border/mutual-NN masking.
"""

import sys
import numpy as np

sys.path.insert(0, "/opt/trn_rl_repo")

import concourse.bacc as bacc
import concourse.bass as bass
import concourse.tile as tile
from concourse import mybir
from concourse.bass_utils import run_bass_kernel_spmd
from concourse import hw_specs as _hw_specs

# Pin every ACT function to the one table set that contains all of
# {exp, ln, square, copy, identity} so the kernel does a single
# ACT_TABLE_LOAD instead of thrashing between sets (2.7us per switch).
_orig_get_activation_tables = _hw_specs.get_activation_tables

def _pinned_activation_tables(module_arch):
    tables = _orig_get_activation_tables(module_arch)
    keep = "natural_log_exp_and_others"
    return {
        name: (funcs if name == keep else set())
        for name, funcs in tables.items()
    }

bacc.get_activation_tables = _pinned_activation_tables

N_CORES = 8
N, L, S, C = 2, 4800, 4800, 256
H0, W0, H1, W1 = 60, 80, 60, 80
THR = 0.2
BORDER_RM = 2
TEMP = 0.1
SCALE = 1.0 / (C * TEMP)  # folded into f0T: (1/sqrt(C))^2 / TEMP

SHARD = L // N_CORES          # 600 rows per core per pair
LP = 120                      # rows per l-tile (partition dim)
NLT = SHARD // LP             # 5 l-tiles
NBW = 480                     # columns per matmul block
NNB = S // NBW                # 10 column blocks
EBW = 960                     # columns per exp/psum superblock (2 banks)
NEB = S // EBW                # 5 exp blocks
TBW = 960                     # columns per conf TT/DMA block
NTB = S // TBW
KC = 128                      # contraction chunk (partitions)
NKC = C // KC                 # 2 chunks

F32 = mybir.dt.float32
F32R = mybir.dt.float32r


def _row_blocks(total):
    blocks = []
    start = 0
    while start < total:
        sz = min(128, total - start)
        blocks.append((start, sz))
        start += sz
    return blocks


def build_nc():
    nc = bacc.Bacc("TRN2", target_bir_lowering=False, num_devices=N_CORES)

    f0s = nc.declare_dram_parameter("f0s", [N, SHARD, C], F32, isOutput=False)
    f1f = nc.declare_dram_parameter("f1f", [N, S, C], F32, isOutput=False)
    ident = nc.declare_dram_parameter("ident", [128, 128], F32, isOutput=False)
    ones_in = nc.declare_dram_parameter("ones_in", [128, 128], F32, isOutput=False)
    conf_o = nc.declare_dram_parameter("conf_o", [N, SHARD, S], F32, isOutput=True)

    with tile.TileContext(nc) as tc:
        with (
            tc.tile_pool(name="single", bufs=1) as single,
            tc.tile_pool(name="f1n", bufs=4) as f1n_pool,
            tc.tile_pool(name="f0n", bufs=2) as f0n_pool,
            tc.tile_pool(name="f1T", bufs=2) as f1T_pool,
            tc.tile_pool(name="f0T", bufs=2) as f0T_pool,
            tc.tile_pool(name="Pp", bufs=6) as P_pool,
            tc.tile_pool(name="stats", bufs=12) as stats_pool,
            tc.tile_pool(name="tiny", bufs=24) as tiny_pool,
            tc.tile_pool(name="cs", bufs=2) as cs_pool,
            tc.tile_pool(name="cb", bufs=1) as cb_pool,
            tc.tile_pool(name="stage", bufs=3) as stage_pool,
            tc.tile_pool(name="pt", bufs=2, space="PSUM") as pt_pool,
            tc.tile_pool(name="ps", bufs=2, space="PSUM") as ps_pool,
            tc.tile_pool(name="pc", bufs=2, space="PSUM") as pc_pool,
            tc.tile_pool(name="dram", bufs=1, space="DRAM") as dram_pool,
        ):
            ident_t = single.tile([128, 128], F32)
            nc.sync.dma_start(out=ident_t, in_=ident[:, :])
            ones_f = single.tile([128, 128], F32)
            nc.sync.dma_start(out=ones_f, in_=ones_in[:, :])
            ones_r = single.tile([128, 128], F32R)
            nc.vector.tensor_copy(ones_r, ones_f)

            cc_in = []
            cc_out = []
            stag = []
            for p in range(N):
                t_in = dram_pool.tile([S], F32, name=f"cc_in{p}", tag=f"cc_in{p}")
                t_out = dram_pool.tile([S], F32, name=f"cc_out{p}", addr_space="Shared", tag=f"cc_out{p}")
                t_st = dram_pool.tile([S], F32, name=f"stag{p}", tag=f"stag{p}")
                cc_in.append(t_in)
                cc_out.append(t_out)
                stag.append(t_st)

            for p in range(N):
                # ---- transpose f0 shard: [SHARD, C] -> f0T[k] [128, SHARD] ----
                f0T = []
                for k in range(NKC):
                    t = f0T_pool.tile([128, SHARD], F32R, name=f"f0T{p}{k}", tag="f0T")
                    f0T.append(t)
                for (rs, sz) in _row_blocks(SHARD):
                    f0nb = f0n_pool.tile([128, C], F32, name="f0nb")
                    nc.sync.dma_start(out=f0nb[:sz, :], in_=f0s[p, rs : rs + sz, :])
                    for k in range(NKC):
                        ptt = pt_pool.tile([128, 512], F32, name="ptt0", tag="ptt")
                        nc.tensor.transpose(
                            ptt[:, :sz],
                            f0nb[:sz, k * KC : (k + 1) * KC],
                            ident_t[:sz, :sz],
                        )
                        # scale folded here; output dtype fp32r rounds
                        nc.scalar.activation(
                            f0T[k][:, rs : rs + sz],
                            ptt[:, :sz],
                            mybir.ActivationFunctionType.Copy,
                            scale=SCALE,
                        )

                # ---- transpose f1: [S, C] -> f1T[k] [128, S] ----
                f1T = []
                for k in range(NKC):
                    t = f1T_pool.tile([128, S], F32R, name=f"f1T{p}{k}", tag="f1T")
                    f1T.append(t)
                f1_blocks = _row_blocks(S)
                for g in range(0, len(f1_blocks), 4):
                    group = f1_blocks[g : g + 4]
                    tiles = []
                    for (rs, sz) in group:
                        f1nb = f1n_pool.tile([128, C], F32, name="f1nb")
                        nc.sync.dma_start(out=f1nb[:sz, :], in_=f1f[p, rs : rs + sz, :])
                        tiles.append((f1nb, rs, sz))
                    for k in range(NKC):
                        ptt = pt_pool.tile([128, 512], F32, name="ptt1", tag="ptt")
                        off = 0
                        for (f1nb, rs, sz) in tiles:
                            nc.tensor.transpose(
                                ptt[:, off : off + sz],
                                f1nb[:sz, k * KC : (k + 1) * KC],
                                ident_t[:sz, :sz],
                            )
                            off += sz
                        gs = group[0][0]
                        nc.vector.tensor_copy(f1T[k][:, gs : gs + off], ptt[:, :off])

                # ---- matmul + exp (phase A) ----
                P_tiles = []
                rsp_tiles = []
                srinv_tiles = []
                for lt in range(NLT):
                    P_lt = P_pool.tile([LP, S], F32R, name=f"P{lt}", tag="P")
                    P_tiles.append(P_lt)
                    rsp = stats_pool.tile([LP, 16], F32, name=f"rsp{lt}", tag="rsp")
                    rsp_tiles.append(rsp)
                    lc = lt * LP
                    for eb in range(NEB):
                        e0 = eb * EBW
                        # two bank-aligned 512-wide regions; 480 cols used each
                        pst = ps_pool.tile([LP, 2, 512], F32, name="pst")
                        for h in range(EBW // NBW):
                            for k in range(NKC):
                                nc.tensor.matmul(
                                    pst[:, h, 0:NBW],
                                    f0T[k][:, lc : lc + LP],
                                    f1T[k][:, e0 + h * NBW : e0 + (h + 1) * NBW],
                                    start=(k == 0),
                                    stop=(k == NKC - 1),
                                )
                        nc.scalar.activation(
                            P_lt[:, e0 : e0 + EBW].rearrange(
                                "p (a b) -> p a b", a=2
                            ),
                            pst[:, :, 0:NBW],
                            mybir.ActivationFunctionType.Exp,
                            accum_out=rsp[:, eb : eb + 1],
                        )
                    # rsum -> srinv = exp(-0.5 * ln(rsum))
                    rs1 = tiny_pool.tile([LP, 1], F32, name=f"rs1_{lt}", tag="rs1")
                    nc.vector.tensor_reduce(
                        rs1, rsp[:, 0:NEB], axis=mybir.AxisListType.X,
                        op=mybir.AluOpType.add,
                    )
                    ln1 = tiny_pool.tile([LP, 1], F32, name=f"ln1_{lt}", tag="ln1")
                    nc.scalar.activation(
                        ln1, rs1, mybir.ActivationFunctionType.Ln
                    )
                    srinv = tiny_pool.tile([LP, 1], F32, name=f"srinv{lt}", tag="srinv")
                    nc.scalar.activation(
                        srinv, ln1, mybir.ActivationFunctionType.Exp, scale=-0.5
                    )
                    srinv_tiles.append(srinv)

                # ---- csum via ones-matmul, accumulate over l-tiles ----
                for nb in range(NNB):
                    cb0 = nb * NBW
                    pct = pc_pool.tile([128, NBW], F32, name="pct")
                    for lt in range(NLT):
                        nc.tensor.matmul(
                            pct,
                            ones_r[:LP, :],
                            P_tiles[lt][:, cb0 : cb0 + NBW],
                            start=(lt == 0),
                            stop=(lt == NLT - 1),
                        )
                    csb = cs_pool.tile([1, NBW], F32, name="csb")
                    nc.scalar.copy(csb, pct[0:1, :])
                    nc.sync.dma_start(out=cc_in[p][cb0 : cb0 + NBW], in_=csb)

                # ---- all-reduce column sums across the 8 cores ----
                nc.gpsimd.collective_compute(
                    "AllReduce",
                    mybir.AluOpType.add,
                    replica_groups=[list(range(N_CORES))],
                    ins=[cc_in[p][:].opt()],
                    outs=[cc_out[p][:].opt()],
                )

                # ---- cinv = 1/csum, broadcast to all partitions ----
                csr = cs_pool.tile([96, 50], F32, name="csr")
                nc.sync.dma_start(
                    out=csr, in_=cc_out[p][:].rearrange("(a b) -> a b", a=96)
                )
                cis = cs_pool.tile([96, 50], F32, name="cis")
                nc.vector.reciprocal(cis, csr)
                nc.sync.dma_start(
                    out=stag[p][:].rearrange("(a b) -> a b", a=96), in_=cis
                )
                cb_t = cb_pool.tile([128, S], F32, name="cb_t")
                stag_ap = stag[p][:]
                stag_bcast = bass.AP(
                    tensor=stag_ap.tensor,
                    offset=stag_ap.offset,
                    ap=[[0, 128]] + list(stag_ap.ap),
                )
                nc.sync.dma_start(out=cb_t, in_=stag_bcast)

                # ---- phase B: conf = (P * srinv)^2 * cinv ----
                for lt in range(NLT):
                    P_lt = P_tiles[lt]
                    # in-place square with per-partition scale (via f32 view;
                    # the DVE/ISA path does not accept fp32r on ACT in-place)
                    nc.scalar.activation(
                        P_lt,
                        P_lt.bitcast(F32),
                        mybir.ActivationFunctionType.Square,
                        scale=srinv_tiles[lt],
                    )
                    lr = lt * LP
                    for tb in range(NTB):
                        cb0 = tb * TBW
                        st = stage_pool.tile([LP, TBW], F32, name="st")
                        nc.vector.tensor_mul(
                            st,
                            P_lt.bitcast(F32)[:, cb0 : cb0 + TBW],
                            cb_t[:LP, cb0 : cb0 + TBW],
                        )
                        nc.sync.dma_start(
                            out=conf_o[p, lr : lr + LP, cb0 : cb0 + TBW], in_=st
                        )

    nc.compile()
    return nc


_CACHED = {}


def _get_nc():
    if "nc" not in _CACHED:
        _CACHED["nc"] = build_nc()
    return _CACHED["nc"]


def run_device(feat_c0, feat_c1, trace=False, tmpdir=None):
    """Runs the SPMD kernel; returns (conf [N,L,S], rowmax [N,L], results obj)."""
    nc = _get_nc()
    ident = np.eye(128, dtype=np.float32)
    ones = np.ones((128, 128), dtype=np.float32)
    in_maps = []
    for c in range(N_CORES):
        rs = c * SHARD
        in_maps.append(
            {
                "f0s": np.ascontiguousarray(feat_c0[:, rs : rs + SHARD, :]),
                "f1f": np.ascontiguousarray(feat_c1),
                "ident": ident,
                "ones_in": ones,
            }
        )
    res = run_bass_kernel_spmd(
        nc, in_maps, list(range(N_CORES)), trace=trace, tmpdir=tmpdir
    )
    conf = np.empty((N, L, S), dtype=np.float32)
    for c in range(N_CORES):
        rs = c * SHARD
        conf[:, rs : rs + SHARD, :] = res.results[c]["conf_o"]
    return conf, res


def _interior(n, idx):
    return (idx >= BORDER_RM) & (idx < n - BORDER_RM)


def kernel(feat_c0, feat_c1, h0c, w0c, h1c, w1c):
    feat_c0 = np.asarray(feat_c0, dtype=np.float32)
    feat_c1 = np.asarray(feat_c1, dtype=np.float32)
    h0c, w0c, h1c, w1c = int(h0c), int(w0c), int(h1c), int(w1c)
    assert feat_c0.shape == (N, L, C) and feat_c1.shape == (N, S, C)
    assert (h0c * w0c, h1c * w1c) == (L, S)

    conf, _ = run_device(feat_c0, feat_c1)
    rowmax = conf.max(axis=2)

    # ---- host finalize: threshold + border + mutual-NN (tiny) ----
    mask_v = np.zeros((N, L), dtype=bool)
    j_ids = np.zeros((N, L), dtype=np.int32)
    mconf = np.zeros((N, L), dtype=np.float32)

    cand_n, cand_l = np.nonzero(rowmax > THR)
    for n_i, l_i in zip(cand_n, cand_l):
        # row border (interior of the h0c x w0c grid)
        if not (_interior(h0c, l_i // w0c) and _interior(w0c, l_i % w0c)):
            continue
        row = conf[n_i, l_i]
        j = int(np.argmax(row))
        v = row[j]
        if not (v > THR):
            continue
        # column border (interior of the h1c x w1c grid)
        if not (_interior(h1c, j // w1c) and _interior(w1c, j % w1c)):
            continue
        # mutual nearest neighbor: also the max of its column
        if conf[n_i, :, j].max() != v:
            continue
        mask_v[n_i, l_i] = True
        j_ids[n_i, l_i] = j
        mconf[n_i, l_i] = v

    mkpts1_c = np.stack([j_ids % w1c, j_ids // w1c], axis=-1).astype(np.int32)
    return conf, mask_v, j_ids, mkpts1_c, mconf


# revision 12
# speedup vs baseline: 1.2349x; 1.0390x over previous
"""CoarseMatching (LoFTR-style dual-softmax matching) on 8 Trainium2 cores.

Sharding: each core owns 600 rows (L dim) of both pairs (N=2).  Per pair:
sim = (f0 @ f1^T) / (C^0.5 * C^0.5 * TEMP) computed with fp32r matmuls,
P = exp(sim) kept resident, row sums via ACT accum, column sums via
ones-matmul on PE + one 8-core AllReduce, conf = (P*rsqrt(rsum))^2 * cinv
written back to HBM.  Host assembles shards and applies the (empty in
practice) threshold/border/mutual-NN masking.
"""

import sys
import numpy as np

sys.path.insert(0, "/opt/trn_rl_repo")

import concourse.bacc as bacc
import concourse.bass as bass
import concourse.tile as tile
from concourse import mybir
from concourse.bass_utils import run_bass_kernel_spmd
from concourse import hw_specs as _hw_specs

# Pin every ACT function to the one table set that contains all of
# {exp, ln, square, copy, identity} so the kernel does a single
# ACT_TABLE_LOAD instead of thrashing between sets (2.7us per switch).
_orig_get_activation_tables = _hw_specs.get_activation_tables

def _pinned_activation_tables(module_arch):
    tables = _orig_get_activation_tables(module_arch)
    keep = "natural_log_exp_and_others"
    return {
        name: (funcs if name == keep else set())
        for name, funcs in tables.items()
    }

bacc.get_activation_tables = _pinned_activation_tables

N_CORES = 8
N, L, S, C = 2, 4800, 4800, 256
H0, W0, H1, W1 = 60, 80, 60, 80
THR = 0.2
BORDER_RM = 2
TEMP = 0.1
SCALE = 1.0 / (C * TEMP)  # folded into f0T: (1/sqrt(C))^2 / TEMP

SHARD = L // N_CORES          # 600 rows per core per pair
LP = 120                      # rows per l-tile (partition dim)
NLT = SHARD // LP             # 5 l-tiles
NBW = 480                     # columns per matmul block
NNB = S // NBW                # 10 column blocks
EBW = 960                     # columns per exp/psum superblock (2 banks)
NEB = S // EBW                # 5 exp blocks
TBW = 960                     # columns per conf TT/DMA block
NTB = S // TBW
KC = 128                      # contraction chunk (partitions)
NKC = C // KC                 # 2 chunks

F32 = mybir.dt.float32
F32R = mybir.dt.float32r


def _row_blocks(total):
    blocks = []
    start = 0
    while start < total:
        sz = min(128, total - start)
        blocks.append((start, sz))
        start += sz
    return blocks


def build_nc():
    nc = bacc.Bacc("TRN2", target_bir_lowering=False, num_devices=N_CORES)

    f0s = nc.declare_dram_parameter("f0s", [N, SHARD, C], F32, isOutput=False)
    f1f = nc.declare_dram_parameter("f1f", [N, S, C], F32, isOutput=False)
    ident = nc.declare_dram_parameter("ident", [128, 128], F32, isOutput=False)
    ones_in = nc.declare_dram_parameter("ones_in", [128, 128], F32, isOutput=False)
    conf_o = nc.declare_dram_parameter("conf_o", [N, SHARD, S], F32, isOutput=True)

    with tile.TileContext(nc) as tc:
        with (
            tc.tile_pool(name="single", bufs=1) as single,
            tc.tile_pool(name="f1n", bufs=4) as f1n_pool,
            tc.tile_pool(name="f0n", bufs=2) as f0n_pool,
            tc.tile_pool(name="f1T", bufs=2) as f1T_pool,
            tc.tile_pool(name="f0T", bufs=2) as f0T_pool,
            tc.tile_pool(name="Pp", bufs=6) as P_pool,
            tc.tile_pool(name="stats", bufs=12) as stats_pool,
            tc.tile_pool(name="tiny", bufs=24) as tiny_pool,
            tc.tile_pool(name="cs", bufs=2) as cs_pool,
            tc.tile_pool(name="cb", bufs=1) as cb_pool,
            tc.tile_pool(name="stage", bufs=3) as stage_pool,
            tc.tile_pool(name="pt", bufs=2, space="PSUM") as pt_pool,
            tc.tile_pool(name="ps", bufs=2, space="PSUM") as ps_pool,
            tc.tile_pool(name="pc", bufs=2, space="PSUM") as pc_pool,
            tc.tile_pool(name="dram", bufs=1, space="DRAM") as dram_pool,
        ):
            ident_t = single.tile([128, 128], F32)
            nc.sync.dma_start(out=ident_t, in_=ident[:, :])
            ones_f = single.tile([128, 128], F32)
            nc.sync.dma_start(out=ones_f, in_=ones_in[:, :])
            ones_r = single.tile([128, 128], F32R)
            nc.vector.tensor_copy(ones_r, ones_f)

            cc_in = []
            cc_out = []
            stag = []
            for p in range(N):
                t_in = dram_pool.tile([S], F32, name=f"cc_in{p}", tag=f"cc_in{p}")
                t_out = dram_pool.tile([S], F32, name=f"cc_out{p}", addr_space="Shared", tag=f"cc_out{p}")
                t_st = dram_pool.tile([S], F32, name=f"stag{p}", tag=f"stag{p}")
                cc_in.append(t_in)
                cc_out.append(t_out)
                stag.append(t_st)

            def load_and_transpose(p):
                """DMA f0/f1 in natural layout and PE-transpose into fp32r."""
                f0T = []
                for k in range(NKC):
                    t = f0T_pool.tile([128, SHARD], F32R, name=f"f0T{p}{k}", tag="f0T")
                    f0T.append(t)
                for (rs, sz) in _row_blocks(SHARD):
                    f0nb = f0n_pool.tile([128, C], F32, name="f0nb")
                    nc.sync.dma_start(out=f0nb[:sz, :], in_=f0s[p, rs : rs + sz, :])
                    for k in range(NKC):
                        ptt = pt_pool.tile([128, 512], F32, name="ptt0", tag="ptt")
                        nc.tensor.transpose(
                            ptt[:, :sz],
                            f0nb[:sz, k * KC : (k + 1) * KC],
                            ident_t[:sz, :sz],
                        )
                        # sim scale folded here; fp32r output rounds
                        nc.scalar.activation(
                            f0T[k][:, rs : rs + sz],
                            ptt[:, :sz],
                            mybir.ActivationFunctionType.Copy,
                            scale=SCALE,
                        )
                f1T = []
                for k in range(NKC):
                    t = f1T_pool.tile([128, S], F32R, name=f"f1T{p}{k}", tag="f1T")
                    f1T.append(t)
                f1_blocks = _row_blocks(S)
                for g in range(0, len(f1_blocks), 4):
                    group = f1_blocks[g : g + 4]
                    tiles = []
                    for (rs, sz) in group:
                        f1nb = f1n_pool.tile([128, C], F32, name="f1nb")
                        nc.sync.dma_start(out=f1nb[:sz, :], in_=f1f[p, rs : rs + sz, :])
                        tiles.append((f1nb, rs, sz))
                    for k in range(NKC):
                        ptt = pt_pool.tile([128, 512], F32, name="ptt1", tag="ptt")
                        off = 0
                        for (f1nb, rs, sz) in tiles:
                            nc.tensor.transpose(
                                ptt[:, off : off + sz],
                                f1nb[:sz, k * KC : (k + 1) * KC],
                                ident_t[:sz, :sz],
                            )
                            off += sz
                        gs = group[0][0]
                        nc.vector.tensor_copy(f1T[k][:, gs : gs + off], ptt[:, :off])
                return f0T, f1T

            def phase_a(p, f0T, f1T):
                """sim matmuls -> exp into resident P tiles + row sums."""
                P_tiles = []
                srinv_tiles = []
                for lt in range(NLT):
                    P_lt = P_pool.tile([LP, S], F32R, name=f"P{lt}", tag="P")
                    P_tiles.append(P_lt)
                    rsp = stats_pool.tile([LP, 16], F32, name=f"rsp{lt}", tag="rsp")
                    lc = lt * LP
                    for eb in range(NEB):
                        e0 = eb * EBW
                        # two bank-aligned 512-wide regions; 480 cols used each
                        pst = ps_pool.tile([LP, 2, 512], F32, name="pst")
                        for h in range(EBW // NBW):
                            for k in range(NKC):
                                nc.tensor.matmul(
                                    pst[:, h, 0:NBW],
                                    f0T[k][:, lc : lc + LP],
                                    f1T[k][:, e0 + h * NBW : e0 + (h + 1) * NBW],
                                    start=(k == 0),
                                    stop=(k == NKC - 1),
                                )
                        nc.scalar.activation(
                            P_lt[:, e0 : e0 + EBW].rearrange(
                                "p (a b) -> p a b", a=2
                            ),
                            pst[:, :, 0:NBW],
                            mybir.ActivationFunctionType.Exp,
                            accum_out=rsp[:, eb : eb + 1],
                        )
                    # rsum -> srinv = exp(-0.5 * ln(rsum))
                    rs1 = tiny_pool.tile([LP, 1], F32, name=f"rs1_{lt}", tag="rs1")
                    nc.vector.tensor_reduce(
                        rs1, rsp[:, 0:NEB], axis=mybir.AxisListType.X,
                        op=mybir.AluOpType.add,
                    )
                    ln1 = tiny_pool.tile([LP, 1], F32, name=f"ln1_{lt}", tag="ln1")
                    nc.scalar.activation(ln1, rs1, mybir.ActivationFunctionType.Ln)
                    srinv = tiny_pool.tile([LP, 1], F32, name=f"srinv{lt}", tag="srinv")
                    nc.scalar.activation(
                        srinv, ln1, mybir.ActivationFunctionType.Exp, scale=-0.5
                    )
                    srinv_tiles.append(srinv)
                return P_tiles, srinv_tiles

            def csum_and_allreduce(p, P_tiles):
                """column sums via ones-matmul, then 8-core AllReduce."""
                for nb in range(NNB):
                    cb0 = nb * NBW
                    pct = pc_pool.tile([128, NBW], F32, name="pct")
                    for lt in range(NLT):
                        nc.tensor.matmul(
                            pct,
                            ones_r[:LP, :],
                            P_tiles[lt][:, cb0 : cb0 + NBW],
                            start=(lt == 0),
                            stop=(lt == NLT - 1),
                        )
                    csb = cs_pool.tile([1, NBW], F32, name="csb")
                    nc.scalar.copy(csb, pct[0:1, :])
                    nc.gpsimd.dma_start(out=cc_in[p][cb0 : cb0 + NBW], in_=csb)
                nc.gpsimd.collective_compute(
                    "AllReduce",
                    mybir.AluOpType.add,
                    replica_groups=[list(range(N_CORES))],
                    ins=[cc_in[p][:].opt()],
                    outs=[cc_out[p][:].opt()],
                )

            def cinv_broadcast(p):
                """cinv = 1/csum on 96 lanes, then DMA-broadcast to 128 parts."""
                csr = cs_pool.tile([96, 50], F32, name="csr")
                nc.gpsimd.dma_start(
                    out=csr, in_=cc_out[p][:].rearrange("(a b) -> a b", a=96)
                )
                cis = cs_pool.tile([96, 50], F32, name="cis")
                nc.vector.reciprocal(cis, csr)
                nc.gpsimd.dma_start(
                    out=stag[p][:].rearrange("(a b) -> a b", a=96), in_=cis
                )
                cb_t = cb_pool.tile([128, S], F32, name="cb_t")
                stag_ap = stag[p][:]
                stag_bcast = bass.AP(
                    tensor=stag_ap.tensor,
                    offset=stag_ap.offset,
                    ap=[[0, 128]] + list(stag_ap.ap),
                )
                nc.gpsimd.dma_start(out=cb_t, in_=stag_bcast)
                return cb_t

            def phase_b(p, P_tiles, srinv_tiles, cb_t):
                """conf = (P * srinv)^2 * cinv -> HBM."""
                for lt in range(NLT):
                    P_lt = P_tiles[lt]
                    nc.scalar.activation(
                        P_lt,
                        P_lt.bitcast(F32),
                        mybir.ActivationFunctionType.Square,
                        scale=srinv_tiles[lt],
                    )
                    lr = lt * LP
                    for tb in range(NTB):
                        cb0 = tb * TBW
                        st = stage_pool.tile([LP, TBW], F32, name="st")
                        nc.vector.tensor_mul(
                            st,
                            P_lt.bitcast(F32)[:, cb0 : cb0 + TBW],
                            cb_t[:LP, cb0 : cb0 + TBW],
                        )
                        nc.sync.dma_start(
                            out=conf_o[p, lr : lr + LP, cb0 : cb0 + TBW], in_=st
                        )

            # Interleave the two pairs so pair-1 load/transpose/compute fills
            # the pair-0 AllReduce + phase-B window (PE stays warm).
            f0T0, f1T0 = load_and_transpose(0)
            P0, sr0 = phase_a(0, f0T0, f1T0)
            csum_and_allreduce(0, P0)
            f0T1, f1T1 = load_and_transpose(1)
            cb0_t = cinv_broadcast(0)
            phase_b(0, P0, sr0, cb0_t)
            P1, sr1 = phase_a(1, f0T1, f1T1)
            csum_and_allreduce(1, P1)
            cb1_t = cinv_broadcast(1)
            phase_b(1, P1, sr1, cb1_t)

    nc.compile()
    return nc


_CACHED = {}


def _get_nc():
    if "nc" not in _CACHED:
        _CACHED["nc"] = build_nc()
    return _CACHED["nc"]


def run_device(feat_c0, feat_c1, trace=False, tmpdir=None):
    """Runs the SPMD kernel; returns (conf [N,L,S], rowmax [N,L], results obj)."""
    nc = _get_nc()
    ident = np.eye(128, dtype=np.float32)
    ones = np.ones((128, 128), dtype=np.float32)
    in_maps = []
    for c in range(N_CORES):
        rs = c * SHARD
        in_maps.append(
            {
                "f0s": np.ascontiguousarray(feat_c0[:, rs : rs + SHARD, :]),
                "f1f": np.ascontiguousarray(feat_c1),
                "ident": ident,
                "ones_in": ones,
            }
        )
    res = run_bass_kernel_spmd(
        nc, in_maps, list(range(N_CORES)), trace=trace, tmpdir=tmpdir
    )
    conf = np.empty((N, L, S), dtype=np.float32)
    for c in range(N_CORES):
        rs = c * SHARD
        conf[:, rs : rs + SHARD, :] = res.results[c]["conf_o"]
    return conf, res


def _interior(n, idx):
    return (idx >= BORDER_RM) & (idx < n - BORDER_RM)


def kernel(feat_c0, feat_c1, h0c, w0c, h1c, w1c):
    feat_c0 = np.asarray(feat_c0, dtype=np.float32)
    feat_c1 = np.asarray(feat_c1, dtype=np.float32)
    h0c, w0c, h1c, w1c = int(h0c), int(w0c), int(h1c), int(w1c)
    assert feat_c0.shape == (N, L, C) and feat_c1.shape == (N, S, C)
    assert (h0c * w0c, h1c * w1c) == (L, S)

    conf, _ = run_device(feat_c0, feat_c1)
    rowmax = conf.max(axis=2)

    # ---- host finalize: threshold + border + mutual-NN (tiny) ----
    mask_v = np.zeros((N, L), dtype=bool)
    j_ids = np.zeros((N, L), dtype=np.int32)
    mconf = np.zeros((N, L), dtype=np.float32)

    cand_n, cand_l = np.nonzero(rowmax > THR)
    for n_i, l_i in zip(cand_n, cand_l):
        # row border (interior of the h0c x w0c grid)
        if not (_interior(h0c, l_i // w0c) and _interior(w0c, l_i % w0c)):
            continue
        row = conf[n_i, l_i]
        j = int(np.argmax(row))
        v = row[j]
        if not (v > THR):
            continue
        # column border (interior of the h1c x w1c grid)
        if not (_interior(h1c, j // w1c) and _interior(w1c, j % w1c)):
            continue
        # mutual nearest neighbor: also the max of its column
        if conf[n_i, :, j].max() != v:
            continue
        mask_v[n_i, l_i] = True
        j_ids[n_i, l_i] = j
        mconf[n_i, l_i] = v

    mkpts1_c = np.stack([j_ids % w1c, j_ids // w1c], axis=-1).astype(np.int32)
    return conf, mask_v, j_ids, mkpts1_c, mconf
